# revision 1
# baseline (speedup 1.0000x reference)
"""Self-contained Trainium2 Bass kernel for a 3-layer DGL-style GCN + NLL loss.

Strategy (8 NeuronCores, SPMD):
  - Nodes are re-labeled into a [chunk][core][window][128] layout: 98 windows
    of 128 node slots per core (12544 slots/core, 12500 real).  Windows are
    grouped into 4 chunks which double as (a) the AllGather chunking between
    layers and (b) the 4 gather sub-tables (each < 32768 rows, so gather
    indices fit in int16).
  - Edges (dst-sorted) are partitioned by dst window; each (window, seg)
    group's h[src] rows are fetched with dma_gather (int16 idx, -1 pads are
    skipped by HW).  Messages are aggregated into a 128-node window via a
    weighted one-hot matmul accumulated in PSUM:
        aggT[D, n] += g[e, D].T @ S_w[e, n],  S_w[e, n] = w_e * 1[dst_e == n]
  - Dense layer: h = relu(aggT.T @ W + b) via a second matmul pair.
  - Layer 3 keeps logits in PSUM and computes the masked NLL tail on-chip;
    each core emits a partial NLL sum, host sums / N.
  - bf16 data plane (tables, S_w, matmuls), f32 PSUM accumulation and f32
    softmax/NLL tail.
"""

import numpy as np

N = 100000
E = 1600000
D = 128
C = 40
NCORES = 8
RPC = 12500            # real nodes per core
WPC = 98               # windows per core
PW = 128               # nodes per window
NPC = WPC * PW         # 12544 slots per core
NP = NCORES * NPC      # 100352 total slots
CH_W = [25, 25, 24, 24]           # windows per chunk
CH_W0 = [0, 25, 50, 74]
CH_ROWS = [w * PW * NCORES for w in CH_W]      # rows per chunk region
CH_BASE = np.concatenate([[0], np.cumsum(CH_ROWS)]).astype(np.int64)
MAX_T_PER_GATHER = 8   # 1024-index cap per dma_gather

LAST_EXEC_NS = None    # populated after each kernel() call when profiling works
LAST_RESULT = None


def _chunk_of_window(w):
    for c in range(4):
        if CH_W0[c] <= w < CH_W0[c] + CH_W[c]:
            return c
    raise AssertionError(w)


CHUNK_OF_W = np.array([_chunk_of_window(w) for w in range(WPC)])


def _slot_rows(node):
    """Global table row for each original node id (vectorized)."""
    node = np.asarray(node, dtype=np.int64)
    k = node // RPC
    off = node % RPC
    w = off // PW
    p = off % PW
    c = CHUNK_OF_W[w]
    return CH_BASE[c] + k * (np.array(CH_W)[c] * PW) + (w - np.array(CH_W0)[c]) * PW + p


def kernel(features, edge_w, W1, b1, W2, b2, W3, b3, src, dst, labels):
    import sys
    for p in ("/opt/trn_rl_repo",):
        if p not in sys.path:
            sys.path.insert(0, p)
    import ml_dtypes
    import concourse.bass as bass
    import concourse.bacc as bacc
    import concourse.mybir as mybir
    import concourse.tile as tile
    from concourse.bass_utils import run_bass_kernel_spmd

    bf16 = mybir.dt.bfloat16
    f32 = mybir.dt.float32
    i16 = mybir.dt.int16

    features = np.asarray(features, dtype=np.float32)
    edge_w = np.asarray(edge_w, dtype=np.float32)
    W1 = np.asarray(W1, dtype=np.float32); b1 = np.asarray(b1, dtype=np.float32)
    W2 = np.asarray(W2, dtype=np.float32); b2 = np.asarray(b2, dtype=np.float32)
    W3 = np.asarray(W3, dtype=np.float32); b3 = np.asarray(b3, dtype=np.float32)
    src = np.asarray(src, dtype=np.int64)
    dst = np.asarray(dst, dtype=np.int64)
    labels = np.asarray(labels, dtype=np.int64)

    # ---------------- host-side graph preprocessing ----------------
    src_row = _slot_rows(src)                  # global table row of each edge's src
    src_seg = np.searchsorted(CH_BASE[1:], src_row, side="right")
    src_idx = (src_row - CH_BASE[src_seg]).astype(np.int64)   # idx within sub-table

    dst_core = dst // RPC
    dst_off = dst % RPC
    dst_win = dst_off // PW
    dst_loc = dst_off % PW

    # group id per edge: (win, seg) within its core
    grp = dst_win * 4 + src_seg
    NG = WPC * 4

    # per-core views (dst is sorted so each core's edges are contiguous)
    core_bounds = np.searchsorted(dst, np.arange(NCORES + 1) * RPC)
    cnt = np.zeros((NCORES, NG), dtype=np.int64)
    order_per_core = []
    for k in range(NCORES):
        s0, s1 = core_bounds[k], core_bounds[k + 1]
        g = grp[s0:s1]
        o = np.argsort(g, kind="stable") + s0
        order_per_core.append(o)
        cnt[k] = np.bincount(g, minlength=NG)

    cnt_max = np.maximum(cnt.max(axis=0), 1)                  # valid idx per gather group
    Tws = -(-cnt_max // PW)                                    # tiles per (w,s), >=1
    Tws = Tws.reshape(WPC, 4)
    Ttot = Tws.sum(axis=1)                                     # tiles per window
    TC = int(Ttot.sum())                                       # total tiles per core
    # column offsets
    ot = np.concatenate([[0], np.cumsum(Ttot)]).astype(np.int64)     # tile offset per window
    idx_cols_ws = (Tws * PW // 16).reshape(-1)                 # int16 cols per (w,s)
    oc = np.concatenate([[0], np.cumsum(idx_cols_ws)]).astype(np.int64)
    IC = int(oc[-1])

    IDX = np.full((NCORES, 128, IC), -1, dtype=np.int16)
    DSTL = np.zeros((NCORES, 128, TC), dtype=np.float32)
    WGT = np.zeros((NCORES, 128, TC), dtype=np.float32)
    for k in range(NCORES):
        o = order_per_core[k]
        e_idx = src_idx[o]
        e_dl = dst_loc[o].astype(np.float32)
        e_w = edge_w[o].astype(np.float32)
        pos = 0
        for w in range(WPC):
            t0 = int(ot[w])
            for s in range(4):
                n = int(cnt[k, w * 4 + s])
                nmax = int(cnt_max[w * 4 + s])
                T = int(Tws[w, s])
                cap = T * PW
                lst = np.full(cap, -1, dtype=np.int16)
                lst[:n] = e_idx[pos:pos + n].astype(np.int16)
                lst[n:nmax] = 0                      # dummy valid rows (w=0 kills them)
                wrapped = lst.reshape(cap // 16, 16).T           # [16, T*8]
                cb = int(oc[w * 4 + s])
                IDX[k, :, cb:cb + cap // 16] = np.tile(wrapped, (8, 1))
                j = np.arange(n)
                DSTL[k, j % PW, t0 + j // PW] = e_dl[pos:pos + n]
                WGT[k, j % PW, t0 + j // PW] = e_w[pos:pos + n]
                pos += n
                t0 += T

    # features table in slot layout
    FEAT = np.zeros((NP, D), dtype=ml_dtypes.bfloat16)
    rows_all = _slot_rows(np.arange(N))
    FEAT[rows_all] = features.astype(ml_dtypes.bfloat16)

    # labels / mask per (core, window, partition)
    LBL = np.zeros((NCORES, 128, WPC), dtype=np.float32)
    MASK = np.zeros((NCORES, 128, WPC), dtype=np.float32)
    nn = np.arange(N)
    kk = nn // RPC
    off = nn % RPC
    LBL[kk, off % PW, off // PW] = labels.astype(np.float32)
    MASK[kk, off % PW, off // PW] = 1.0

    W1b = W1.astype(ml_dtypes.bfloat16)
    W2b = W2.astype(ml_dtypes.bfloat16)
    W3b = W3.astype(ml_dtypes.bfloat16)
    B1b = b1.reshape(1, -1).astype(ml_dtypes.bfloat16)
    B2b = b2.reshape(1, -1).astype(ml_dtypes.bfloat16)
    B3b = b3.reshape(1, -1).astype(ml_dtypes.bfloat16)

    # ---------------- bass program ----------------
    nc = bacc.Bacc("TRN2", target_bir_lowering=False, debug=False,
                   num_devices=NCORES, num_swdge_queues=4)

    feat_t = nc.dram_tensor("feat", [NP, D], bf16, kind="ExternalInput")
    idx_t = nc.dram_tensor("idx", [128, IC], i16, kind="ExternalInput")
    dstl_t = nc.dram_tensor("dstl", [128, TC], f32, kind="ExternalInput")
    wgt_t = nc.dram_tensor("wgt", [128, TC], f32, kind="ExternalInput")
    lbl_t = nc.dram_tensor("lbl", [128, WPC], f32, kind="ExternalInput")
    mask_t = nc.dram_tensor("mask", [128, WPC], f32, kind="ExternalInput")
    w1_t = nc.dram_tensor("w1", [D, D], bf16, kind="ExternalInput")
    w2_t = nc.dram_tensor("w2", [D, D], bf16, kind="ExternalInput")
    w3_t = nc.dram_tensor("w3", [D, C], bf16, kind="ExternalInput")
    b1_t = nc.dram_tensor("bb1", [1, D], bf16, kind="ExternalInput")
    b2_t = nc.dram_tensor("bb2", [1, D], bf16, kind="ExternalInput")
    b3_t = nc.dram_tensor("bb3", [1, C], bf16, kind="ExternalInput")
    out_t = nc.dram_tensor("out", [1, 1], f32, kind="ExternalOutput")

    Tmax = int(Ttot.max())

    def bcast_ap(ap, inner):
        """append a step-0 inner dim of size `inner` to a [128, T] slice"""
        return bass.AP(ap.tensor, ap.offset, list(ap.ap) + [[0, inner]])

    def rep_ap(ap, times):
        """insert a step-0 middle dim (repeat whole [128,128] tile) -> [128,times,128]"""
        return bass.AP(ap.tensor, ap.offset, [ap.ap[0], [0, times], ap.ap[1]])

    with tile.TileContext(nc) as tc:
        with (
            tc.tile_pool(name="const", bufs=1) as cpool,
            tc.tile_pool(name="gb", bufs=3) as gpool,
            tc.tile_pool(name="sw", bufs=2) as swpool,
            tc.tile_pool(name="small", bufs=2) as spool,
            tc.tile_pool(name="nll", bufs=2) as npool,
            tc.tile_pool(name="ps_agg", bufs=2, space="PSUM") as ps_agg,
            tc.tile_pool(name="ps_h", bufs=2, space="PSUM") as ps_h,
            tc.tile_pool(name="dram", bufs=1, space="DRAM") as dram,
        ):
            # ---- resident metadata ----
            idx_s = cpool.tile([128, IC], i16)
            dstl_s = cpool.tile([128, TC], f32)
            wgt_s = cpool.tile([128, TC], f32)
            lbl_s = cpool.tile([128, WPC], f32)
            mask_s = cpool.tile([128, WPC], f32)
            nc.sync.dma_start(out=idx_s[:], in_=idx_t[:])
            nc.sync.dma_start(out=dstl_s[:], in_=dstl_t[:])
            nc.sync.dma_start(out=wgt_s[:], in_=wgt_t[:])
            nc.sync.dma_start(out=lbl_s[:], in_=lbl_t[:])
            nc.sync.dma_start(out=mask_s[:], in_=mask_t[:])
            w_s = [cpool.tile([D, D], bf16, tag="w1", name="w1s"),
                   cpool.tile([D, D], bf16, tag="w2", name="w2s"),
                   cpool.tile([D, C], bf16, tag="w3", name="w3s")]
            nc.sync.dma_start(out=w_s[0][:], in_=w1_t[:])
            nc.sync.dma_start(out=w_s[1][:], in_=w2_t[:])
            nc.sync.dma_start(out=w_s[2][:], in_=w3_t[:])
            b_s = [cpool.tile([1, D], bf16, tag="b1", name="b1s"),
                   cpool.tile([1, D], bf16, tag="b2", name="b2s"),
                   cpool.tile([1, C], bf16, tag="b3", name="b3s")]
            nc.sync.dma_start(out=b_s[0][:], in_=b1_t[:])
            nc.sync.dma_start(out=b_s[1][:], in_=b2_t[:])
            nc.sync.dma_start(out=b_s[2][:], in_=b3_t[:])

            iota_s = cpool.tile([128, 128], bf16)
            nc.gpsimd.iota(iota_s[:], pattern=[[1, 128]], base=0,
                           channel_multiplier=0,
                           allow_small_or_imprecise_dtypes=True)
            iota40 = cpool.tile([128, C], f32)
            nc.gpsimd.iota(iota40[:], pattern=[[1, C]], base=0,
                           channel_multiplier=0,
                           allow_small_or_imprecise_dtypes=True)
            ones1 = cpool.tile([1, 128], bf16)
            nc.vector.memset(ones1[:], 1.0)
            onescol = cpool.tile([128, 1], f32)
            nc.vector.memset(onescol[:], 1.0)
            nll_acc = cpool.tile([128, 1], f32)
            nc.vector.memset(nll_acc[:], 0.0)

            # zero-fill gather slots once (stale-NaN protection)
            for zi in range(3):
                t = gpool.tile([128, Tmax, D], bf16, tag="g", name=f"gz{zi}")
                nc.vector.memset(t[:], 0.0)

            # ---- inter-layer DRAM tables ----
            h_mine = [[dram.tile([CH_W[c] * PW, D], bf16, tag=f"hm{l}{c}",
                                 name=f"hm{l}{c}")
                       for c in range(4)] for l in range(2)]
            h_full = [[dram.tile([CH_ROWS[c], D], bf16, tag=f"hf{l}{c}",
                                 name=f"hf{l}{c}", addr_space="Shared")
                       for c in range(4)] for l in range(2)]

            qcounter = [0]

            def do_window(w, table_aps, layer):
                """table_aps: list of 4 APs (sub-tables). layer: 0,1,2"""
                Tw = int(Ttot[w])
                t0 = int(ot[w])
                g = gpool.tile([128, Tmax, D], bf16, tag="g", name="g")
                toff = 0
                for s in range(4):
                    T = int(Tws[w, s])
                    nmax = int(cnt_max[w * 4 + s])
                    cb = int(oc[w * 4 + s])
                    tt = 0
                    while tt < T:
                        tn = min(MAX_T_PER_GATHER, T - tt)
                        nidx = tn * PW
                        nvalid = max(0, min(nidx, nmax - tt * PW))
                        nc.gpsimd.dma_gather(
                            g[:, toff + tt: toff + tt + tn, :],
                            table_aps[s],
                            idx_s[:, cb + tt * 8: cb + tt * 8 + nidx // 16],
                            nidx, nvalid, D,
                            queue_num=qcounter[0] % 4,
                        )
                        qcounter[0] += 1
                        tt += tn
                    toff += T
                # weighted one-hot S_w for the whole window
                s01 = swpool.tile([128, Tmax, 128], bf16, tag="s01")
                swt = swpool.tile([128, Tmax, 128], bf16, tag="swt")
                nc.vector.tensor_tensor(
                    out=s01[:, :Tw, :],
                    in0=rep_ap(iota_s[:], Tw),
                    in1=bcast_ap(dstl_s[:, t0:t0 + Tw], 128),
                    op=mybir.AluOpType.is_equal,
                )
                nc.vector.tensor_tensor(
                    out=swt[:, :Tw, :],
                    in0=s01[:, :Tw, :],
                    in1=bcast_ap(wgt_s[:, t0:t0 + Tw], 128),
                    op=mybir.AluOpType.mult,
                )
                # SpMM accumulation: aggT[D, n] += g_t.T @ S_w_t
                agg = ps_agg.tile([128, 128], f32)
                for t in range(Tw):
                    nc.tensor.matmul(
                        out=agg[:],
                        lhsT=g[:, t, :],
                        rhs=swt[:, t, :],
                        start=(t == 0),
                        stop=(t == Tw - 1),
                    )
                aggT_sb = spool.tile([128, 128], bf16, tag="aggT")
                nc.scalar.copy(aggT_sb[:], agg[:])
                Dout = C if layer == 2 else D
                ph = ps_h.tile([128, Dout], f32)
                nc.tensor.matmul(out=ph[:], lhsT=aggT_sb[:], rhs=w_s[layer][:],
                                 start=True, stop=False)
                nc.tensor.matmul(out=ph[:], lhsT=ones1[:], rhs=b_s[layer][:],
                                 start=False, stop=True)
                if layer < 2:
                    ht = spool.tile([128, D], bf16, tag="ht")
                    nc.scalar.activation(ht[:], ph[:],
                                         mybir.ActivationFunctionType.Relu)
                    c = int(CHUNK_OF_W[w])
                    r0 = (w - CH_W0[c]) * PW
                    nc.sync.dma_start(out=h_mine[layer][c][r0:r0 + PW, :],
                                      in_=ht[:])
                else:
                    # fused masked-NLL tail (f32)
                    mx = npool.tile([128, 1], f32, tag="mx")
                    nc.vector.tensor_reduce(out=mx[:], in_=ph[:],
                                            axis=mybir.AxisListType.X,
                                            op=mybir.AluOpType.max)
                    negmx = npool.tile([128, 1], f32, tag="negmx")
                    nc.vector.tensor_scalar_mul(negmx[:], mx[:], -1.0)
                    expb = npool.tile([128, C], f32, tag="expb")
                    sumexp = npool.tile([128, 1], f32, tag="sumexp")
                    nc.scalar.activation(expb[:], ph[:],
                                         mybir.ActivationFunctionType.Exp,
                                         bias=negmx[:, 0:1],
                                         accum_out=sumexp[:])
                    lse = npool.tile([128, 1], f32, tag="lse")
                    nc.scalar.activation(lse[:], sumexp[:],
                                         mybir.ActivationFunctionType.Ln)
                    junk = npool.tile([128, C], f32, tag="junk")
                    picked = npool.tile([128, 1], f32, tag="picked")
                    nc.vector.scalar_tensor_tensor(
                        out=junk[:], in0=iota40[:],
                        scalar=lbl_s[:, w:w + 1],
                        in1=ph[:],
                        op0=mybir.AluOpType.is_equal,
                        op1=mybir.AluOpType.mult,
                        accum_out=picked[:])
                    t1 = npool.tile([128, 1], f32, tag="t1")
                    nc.vector.tensor_tensor(out=t1[:], in0=lse[:], in1=negmx[:],
                                            op=mybir.AluOpType.subtract)
                    t2 = npool.tile([128, 1], f32, tag="t2")
                    nc.vector.tensor_tensor(out=t2[:], in0=t1[:], in1=picked[:],
                                            op=mybir.AluOpType.subtract)
                    nc.vector.scalar_tensor_tensor(
                        out=nll_acc[:], in0=t2[:],
                        scalar=mask_s[:, w:w + 1],
                        in1=nll_acc[:],
                        op0=mybir.AluOpType.mult,
                        op1=mybir.AluOpType.add)

            # ---------------- the three layers ----------------
            feat_tabs = [feat_t[int(CH_BASE[s]):int(CH_BASE[s + 1]), :]
                         for s in range(4)]
            rg = [list(range(NCORES))]
            import os
            dbg = os.environ.get("GCN_DEBUG", "")
            n_layers = {"L1": 1, "L1AG": 1, "L12": 2}.get(dbg, 3)
            use_ag = dbg != "L1"
            for layer in range(n_layers):
                if layer == 0:
                    tabs = feat_tabs
                else:
                    tabs = [h_full[layer - 1][s][:] for s in range(4)]
                for c in range(4):
                    for w in range(CH_W0[c], CH_W0[c] + CH_W[c]):
                        do_window(w, tabs, layer)
                    if layer < 2 and use_ag and layer < n_layers:
                        nc.gpsimd.collective_compute(
                            "AllGather", mybir.AluOpType.bypass,
                            replica_groups=rg,
                            ins=[h_mine[layer][c].opt()],
                            outs=[h_full[layer][c].opt()],
                        )

            # ---------------- final partial-sum ----------------
            pscalar = ps_h.tile([1, 1], f32, tag="pscalar")
            nc.tensor.matmul(out=pscalar[:], lhsT=nll_acc[:], rhs=onescol[:],
                             start=True, stop=True)
            res_sb = spool.tile([1, 1], f32, tag="res")
            nc.scalar.copy(res_sb[:], pscalar[:])
            nc.sync.dma_start(out=out_t[:], in_=res_sb[:])

    nc.compile()

    in_maps = []
    for k in range(NCORES):
        in_maps.append({
            "feat": FEAT, "idx": IDX[k], "dstl": DSTL[k], "wgt": WGT[k],
            "lbl": LBL[k], "mask": MASK[k],
            "w1": W1b, "w2": W2b, "w3": W3b,
            "bb1": B1b, "bb2": B2b, "bb3": B3b,
        })
    trace_ok = False
    try:
        from antenv.axon_hooks import get_axon_ntff_profile_hook
        trace_ok = get_axon_ntff_profile_hook() is not None
    except Exception:
        pass
    res = run_bass_kernel_spmd(nc, in_maps, list(range(NCORES)), trace=trace_ok)
    global LAST_EXEC_NS, LAST_RESULT
    LAST_EXEC_NS = res.exec_time_ns
    LAST_RESULT = res
    total = sum(float(res.results[k]["out"][0, 0]) for k in range(NCORES))
    return np.float32(total / N)

